# revision 3
# baseline (speedup 1.0000x reference)
"""AdaptiveMixGNNLayer distributed Trainium2 kernel (8 NeuronCores).

out = relu(alpha * (S_LP @ x) @ W_LP^T + (1-alpha) * (S_HP @ x) @ W_HP^T + bias)

Strategy (SPMD, one program on all 8 cores; only input data differs per core):
  - Destination rows are sharded across the 8 cores (6250 rows each); each
    core owns the edges whose destination row falls in its range (rows are
    sorted, so per-core edges are a contiguous slice of each edge array).
  - Rows are greedy-packed into blocks of <= 128 rows such that each block
    has <= T0*128 edges in each set; all cores are padded to the same block
    count (uniform SPMD program).
  - Source-feature staging: the host stages x (cast to bf16) in per-core
    *slab* layout: for each block, the lane-ordered rows x[col_e] of the lp
    tiles then the hp tiles are laid out contiguously, partition-major
    ([128 lane, 2*T0 tile, 128 feat]).  The device streams one slab per
    block with large fully-affine DMAs at HBM bandwidth - no per-edge
    descriptors.  This is a value-blind, row-granular rearrangement of x
    derived from the graph structure only.
  - The aggregation matrices A[e, r] = val[e] * (row_rel[e] == r) for ALL
    2*T0 tiles of a block are built on-chip by TWO batched DVE
    tensor_tensor instructions over a [128, w, 2*T0] layout:
       pass 1: a = (iota_rep == rr_bc)      (is_equal)
       pass 2: a = a * val_bc               (mult)
    iota_rep[p, i, k] = i is a host-shipped constant; rr/val are broadcast
    along the middle (w) axis with stride-0 APs.  Keeping the LAST axis the
    stride-1 tile axis keeps every operand last-dim-contiguous 2-byte, so
    the DVE 2x_1p fast path stays active; the per-instruction SBUF-access
    bubble (~140 ns) is amortized over 2*T0 tiles.  This replaces both the
    previous per-tile tensor_scalar builds (~227 ns/tile) and the
    DMA-streamed prebuilt A tiles (~17 MB/core of HBM traffic).
  - TensorE accumulates aggT[f, r] += G^T @ A into PSUM over the block's
    tiles (G = slab edge features, edge-major [128e, 128f]; the matmul's
    moving operand A[:, :w, t] reads columns strided by 2*T0 - PE cost is
    per-column and does not care).  alpha is folded into the edge values on
    the host.
  - Block epilogue: aggT -> SBUF bf16 (ScalarE copy), psum2 = W_LP^T.T @
    aggT_lp + W_HP^T.T @ aggT_hp (all-bf16 matmuls, 1 cycle/row) in one
    PSUM bank, out^T = relu(psum2 + bias) on ScalarE (bf16 output, cast
    back to f32 on host), DMA the [128o, <=128r] block to DRAM.
  - Host unshards the per-core [nblk, 128o, 128r] outputs back to [N, 128].
"""

import os
import numpy as np

N_NODES = 50000
N_EDGES = 640000
D = 128
NCORES = 8
ROWS_PER_CORE = N_NODES // NCORES  # 6250

_COMPILED = {}


def _plan_blocks(lp_rows, hp_rows, cap):
    """Greedy-pack destination rows into blocks of <=128 rows such that each
    block's edge count stays <= cap in each of the two sets.  All cores are
    padded to the same block count by splitting the largest blocks.  Returns
    per-core lists of (r_start, r_end) relative to the core.
    """
    c_lp = np.bincount(np.asarray(lp_rows), minlength=N_NODES)
    c_hp = np.bincount(np.asarray(hp_rows), minlength=N_NODES)
    grp = np.stack([c_lp, c_hp], axis=1)  # [N, 2]

    plans = []
    for c in range(NCORES):
        r0 = c * ROWS_PER_CORE
        blocks = []
        start = 0
        cnt = np.zeros(2, np.int64)
        for r in range(ROWS_PER_CORE):
            add = grp[r0 + r]
            if (r - start) >= 128 or np.any(cnt + add > cap):
                blocks.append((start, r))
                start = r
                cnt = add.copy()
            else:
                cnt += add
        blocks.append((start, ROWS_PER_CORE))
        plans.append(blocks)

    nblk = max(len(b) for b in plans)
    for c in range(NCORES):
        blocks = plans[c]
        while len(blocks) < nblk:
            widths = [e - st for st, e in blocks]
            i = int(np.argmax(widths))
            st, e = blocks[i]
            mid = st + (e - st) // 2
            blocks[i:i + 1] = [(st, mid), (mid, e)]
        plans[c] = blocks
    return plans, nblk


def _prep_set(rows, cols, vals, plans, nblk, T0):
    """Partition one edge set by destination-row block.

    Returns (rr, val, lanecol):
      rr:      [NCORES, 128, nblk*T0] f32; rr[c, p, b*T0+t] = relative dest
               row of the edge at lane p of tile t of block b (0 for pads)
      val:     same layout, edge value (0 for pads)
      lanecol: [NCORES, nblk*T0*128] int32 source column per lane (0 = pads)
    """
    rows = np.asarray(rows)
    cols = np.asarray(cols)
    vals = np.asarray(vals, np.float32)

    NT = nblk * T0
    rr = np.zeros((NCORES, 128, NT), dtype=np.float32)
    val = np.zeros((NCORES, 128, NT), dtype=np.float32)
    lanecol = np.zeros((NCORES, NT * 128), dtype=np.int32)

    core_bounds = np.searchsorted(rows, np.arange(NCORES + 1) * ROWS_PER_CORE)
    for c in range(NCORES):
        e0, e1 = core_bounds[c], core_bounds[c + 1]
        r = rows[e0:e1] - c * ROWS_PER_CORE
        bounds = [st for st, _ in plans[c]] + [ROWS_PER_CORE]
        bb = np.searchsorted(r, bounds)
        for b in range(nblk):
            s, e = e0 + bb[b], e0 + bb[b + 1]
            n = e - s
            assert n <= T0 * 128, (c, b, n)
            if n == 0:
                continue
            j = np.arange(n)
            brow = (rows[s:e] - c * ROWS_PER_CORE - plans[c][b][0])
            rr[c, j % 128, b * T0 + j // 128] = brow.astype(np.float32)
            val[c, j % 128, b * T0 + j // 128] = vals[s:e]
            lanecol[c, b * T0 * 128 + j] = cols[s:e]
    return rr, val, lanecol


def _build(nblk, T0, rmax):
    import concourse.bacc as bacc
    import concourse.mybir as mybir
    import concourse.tile as tile

    f32 = mybir.dt.float32
    bf16 = mybir.dt.bfloat16

    nc = bacc.Bacc("TRN2", target_bir_lowering=False)

    T2 = 2 * T0
    slab_t = nc.dram_tensor("slab", [128, nblk * T2, 128], bf16,
                            kind="ExternalInput")
    rr_t = nc.dram_tensor("rrcat", [128, nblk * T2], bf16, kind="ExternalInput")
    val_t = nc.dram_tensor("valcat", [128, nblk * T2], bf16,
                           kind="ExternalInput")
    iota_t = nc.dram_tensor("iota_rep", [128, 128, T2], bf16,
                            kind="ExternalInput")
    wlpT_t = nc.dram_tensor("wlpT", [D, D], bf16, kind="ExternalInput")
    whpT_t = nc.dram_tensor("whpT", [D, D], bf16, kind="ExternalInput")
    bias_t = nc.dram_tensor("bias", [128, 1], f32, kind="ExternalInput")
    out_t = nc.dram_tensor("out", [nblk, 128, 128], bf16, kind="ExternalOutput")

    with tile.TileContext(nc) as tc:
        with (
            tc.tile_pool(name="const", bufs=1) as cpool,
            tc.tile_pool(name="gbuf", bufs=10) as gpool,
            tc.tile_pool(name="abuf", bufs=6) as apool,
            tc.tile_pool(name="cagg", bufs=6) as caggpool,
            tc.tile_pool(name="osb", bufs=4) as opool,
            tc.tile_pool(name="psagg", bufs=2, space="PSUM") as psagg,
            tc.tile_pool(name="ps2", bufs=2, space="PSUM") as ps2,
        ):
            rr_sb = cpool.tile_from(rr_t[:], name="rrcat")
            val_sb = cpool.tile_from(val_t[:], name="valcat")
            iota_sb = cpool.tile_from(iota_t[:], name="iota_rep")
            wlpT = cpool.tile_from(wlpT_t[:], name="wlpT")
            whpT = cpool.tile_from(whpT_t[:], name="whpT")
            bias = cpool.tile_from(bias_t[:], name="bias")

            for b in range(nblk):
                w = rmax[b]
                g = gpool.tile([128, T2, 128], bf16, tag="g")
                nc.sync.dma_start(g[:], slab_t[:, b * T2 : (b + 1) * T2, :])

                a_t = apool.tile([128, 128, T2], bf16, tag="A")
                rr_bc = rr_sb[:, None, b * T2 : (b + 1) * T2].broadcast_to(
                    [128, w, T2])
                val_bc = val_sb[:, None, b * T2 : (b + 1) * T2].broadcast_to(
                    [128, w, T2])
                nc.vector.tensor_tensor(
                    a_t[:, :w, :], iota_sb[:, :w, :], rr_bc,
                    mybir.AluOpType.is_equal)
                nc.vector.tensor_tensor(
                    a_t[:, :w, :], a_t[:, :w, :], val_bc,
                    mybir.AluOpType.mult)

                caggs = {}
                for s, off in (("lp", 0), ("hp", T0)):
                    aggT = psagg.tile([128, 128], f32, tag=f"aggT_{s}")
                    for t in range(T0):
                        nc.tensor.matmul(
                            aggT[:, :w],
                            g[:, off + t, :],
                            a_t[:, :w, off + t],
                            start=(t == 0),
                            stop=(t == T0 - 1),
                        )
                    cagg = caggpool.tile([128, 128], bf16, tag=f"cagg_{s}")
                    nc.scalar.copy(cagg[:, :w], aggT[:, :w])
                    caggs[s] = cagg

                psum2 = ps2.tile([128, 128], f32, tag="psum2")
                nc.tensor.matmul(psum2[:, :w], wlpT[:], caggs["lp"][:, :w],
                                 start=True, stop=False)
                nc.tensor.matmul(psum2[:, :w], whpT[:], caggs["hp"][:, :w],
                                 start=False, stop=True)
                osb = opool.tile([128, 128], bf16, tag="osb")
                nc.scalar.activation(
                    osb[:, :w], psum2[:, :w],
                    mybir.ActivationFunctionType.Relu,
                    bias=bias[:, 0:1],
                )
                nc.scalar.dma_start(out_t[b, :, :w], osb[:, :w])

    nc.compile()
    return nc


def kernel(x, lp_rows, lp_cols, lp_vals, hp_rows, hp_cols, hp_vals,
           W_LP, W_HP, bias, alpha_raw):
    import ml_dtypes
    from concourse.bass_utils import run_bass_kernel_spmd

    x = np.asarray(x, dtype=np.float32)
    alpha = 1.0 / (1.0 + np.exp(-float(np.asarray(alpha_raw).reshape(-1)[0])))

    T0 = int(os.environ.get("K2_T0", "12"))

    plans, nblk = _plan_blocks(lp_rows, hp_rows, T0 * 128)
    rmax = tuple(max(plans[c][b][1] - plans[c][b][0] for c in range(NCORES))
                 for b in range(nblk))
    rr_lp, val_lp, lc_lp = _prep_set(
        lp_rows, lp_cols, np.asarray(lp_vals, np.float32) * np.float32(alpha),
        plans, nblk, T0)
    rr_hp, val_hp, lc_hp = _prep_set(
        hp_rows, hp_cols,
        np.asarray(hp_vals, np.float32) * np.float32(1.0 - alpha),
        plans, nblk, T0)

    bf = ml_dtypes.bfloat16
    xbf = np.ascontiguousarray(x.astype(bf))
    wlpT = np.ascontiguousarray(np.asarray(W_LP, np.float32).T.astype(bf))
    whpT = np.ascontiguousarray(np.asarray(W_HP, np.float32).T.astype(bf))
    bias_col = np.ascontiguousarray(np.asarray(bias, np.float32).reshape(128, 1))
    T2 = 2 * T0
    iota_rep = np.ascontiguousarray(np.broadcast_to(
        np.arange(128, dtype=np.float32)[None, :, None],
        (128, 128, T2)).astype(bf))

    NT = nblk * T0

    def cat_meta(m_lp, m_hp):
        # [128, nblk*T0] x2 -> [128, nblk*2T0] with per-block lp then hp
        a = m_lp.reshape(128, nblk, T0)
        b = m_hp.reshape(128, nblk, T0)
        return np.ascontiguousarray(
            np.concatenate([a, b], axis=2).reshape(128, nblk * T2).astype(bf))

    def slabcat(lcl, lch):
        # lane cols [NT*128] x2 -> gathered x rows in [128, nblk*2T0, 128]
        a = lcl.reshape(nblk, T0 * 128)
        b = lch.reshape(nblk, T0 * 128)
        lanes = np.concatenate([a, b], axis=1).reshape(nblk * T2, 128)
        g = xbf[lanes]                         # [nblk*2T0, 128, 128]
        return np.ascontiguousarray(g.transpose(1, 0, 2))

    in_maps = []
    for c in range(NCORES):
        m = {
            "slab": slabcat(lc_lp[c], lc_hp[c]),
            "rrcat": cat_meta(rr_lp[c], rr_hp[c]),
            "valcat": cat_meta(val_lp[c], val_hp[c]),
            "iota_rep": iota_rep, "wlpT": wlpT, "whpT": whpT,
            "bias": bias_col,
        }
        in_maps.append(m)

    key = (nblk, T0, rmax)
    trace = bool(int(os.environ.get("KERNEL_TRACE", "0")))
    res = None
    last_exc = None
    # Rarely the device comes up in a bad state and an execution fails; retry.
    for attempt in range(3):
        if key not in _COMPILED:
            _COMPILED[key] = _build(*key)
        try:
            res = run_bass_kernel_spmd(
                _COMPILED[key], in_maps, list(range(NCORES)), trace=trace)
            break
        except Exception as e:  # noqa: BLE001
            last_exc = e
    if res is None:
        raise last_exc
    kernel.last_result = res

    out = np.empty((N_NODES, D), dtype=np.float32)
    for c in range(NCORES):
        oc = np.asarray(res.results[c]["out"], dtype=np.float32)
        base = c * ROWS_PER_CORE
        for b, (r0, r1) in enumerate(plans[c]):
            out[base + r0 : base + r1, :] = oc[b, :, : r1 - r0].T
    return out


# revision 8
# speedup vs baseline: 1.5895x; 1.5895x over previous
"""AdaptiveMixGNNLayer distributed Trainium2 kernel (8 NeuronCores).

out = relu(alpha * (S_LP @ x) @ W_LP^T + (1-alpha) * (S_HP @ x) @ W_HP^T + bias)

Strategy (SPMD, one program on all 8 cores; only input data differs per core):
  - Destination rows are sharded across the 8 cores (6250 rows each); each
    core owns the edges whose destination row falls in its range (rows are
    sorted, so per-core edges are a contiguous slice of each edge array).
  - Rows are greedy-packed into blocks of <= 128 rows such that each block
    has <= T0*128 edges in each set; all cores are padded to the same block
    count (uniform SPMD program).
  - Source-feature staging: the host stages x (cast to bf16) in per-core
    *slab* layout: for each block, the lane-ordered rows x[col_e] of the lp
    tiles then the hp tiles are laid out contiguously, partition-major
    ([128 lane, 2*T0 tile, 128 feat]).  The device streams one slab per
    block with large fully-affine DMAs at HBM bandwidth - no per-edge
    descriptors.  This is a value-blind, row-granular rearrangement of x
    derived from the graph structure only.
  - The aggregation matrices A[e, r] = val[e] * (row_rel[e] == r) for ALL
    2*T0 tiles of a block are built on-chip by TWO batched DVE
    tensor_tensor instructions over a [128, w, 2*T0] layout:
       pass 1: a = (iota_rep == rr_bc)      (is_equal)
       pass 2: a = a * val_bc               (mult)
    iota_rep[p, i, k] = i is a host-shipped constant; rr/val are broadcast
    along the middle (w) axis with stride-0 APs.  Keeping the LAST axis the
    stride-1 tile axis keeps every operand last-dim-contiguous 2-byte, so
    the DVE 2x_1p fast path stays active; the per-instruction SBUF-access
    bubble (~140 ns) is amortized over 2*T0 tiles.  This replaces both the
    previous per-tile tensor_scalar builds (~227 ns/tile) and the
    DMA-streamed prebuilt A tiles (~17 MB/core of HBM traffic).
  - TensorE accumulates aggT[f, r] += G^T @ A into PSUM over the block's
    tiles (G = slab edge features, edge-major [128e, 128f]; the matmul's
    moving operand A[:, :w, t] reads columns strided by 2*T0 - PE cost is
    per-column and does not care).  alpha is folded into the edge values on
    the host.
  - Block epilogue: aggT -> SBUF bf16 (ScalarE copy), psum2 = W_LP^T.T @
    aggT_lp + W_HP^T.T @ aggT_hp (all-bf16 matmuls, 1 cycle/row) in one
    PSUM bank, out^T = relu(psum2 + bias) on ScalarE (bf16 output, cast
    back to f32 on host), DMA the [128o, <=128r] block to DRAM.
  - Host unshards the per-core [nblk, 128o, 128r] outputs back to [N, 128].
"""

import os
import numpy as np

N_NODES = 50000
N_EDGES = 640000
D = 128
NCORES = 8
ROWS_PER_CORE = N_NODES // NCORES  # 6250

_COMPILED = {}


def _plan_blocks(lp_rows, hp_rows, cap, rcap):
    """Greedy-pack destination rows into blocks of <=rcap rows such that each
    block's edge count stays <= cap in each of the two sets.  All cores are
    padded to the same block count by splitting the largest blocks.  Returns
    per-core lists of (r_start, r_end) relative to the core.
    """
    c_lp = np.bincount(np.asarray(lp_rows), minlength=N_NODES)
    c_hp = np.bincount(np.asarray(hp_rows), minlength=N_NODES)
    grp = np.stack([c_lp, c_hp], axis=1)  # [N, 2]

    plans = []
    for c in range(NCORES):
        r0 = c * ROWS_PER_CORE
        blocks = []
        start = 0
        cnt = np.zeros(2, np.int64)
        for r in range(ROWS_PER_CORE):
            add = grp[r0 + r]
            if (r - start) >= rcap or np.any(cnt + add > cap):
                blocks.append((start, r))
                start = r
                cnt = add.copy()
            else:
                cnt += add
        blocks.append((start, ROWS_PER_CORE))
        plans.append(blocks)

    nblk = max(len(b) for b in plans)
    for c in range(NCORES):
        blocks = plans[c]
        while len(blocks) < nblk:
            widths = [e - st for st, e in blocks]
            i = int(np.argmax(widths))
            st, e = blocks[i]
            mid = st + (e - st) // 2
            blocks[i:i + 1] = [(st, mid), (mid, e)]
        plans[c] = blocks
    return plans, nblk


def _prep_set(rows, cols, vals, plans, nblk, T0):
    """Partition one edge set by destination-row block.

    Returns (rr, val, lanecol):
      rr:      [NCORES, 128, nblk*T0] f32; rr[c, p, b*T0+t] = relative dest
               row of the edge at lane p of tile t of block b (0 for pads)
      val:     same layout, edge value (0 for pads)
      lanecol: [NCORES, nblk*T0*128] int32 source column per lane (0 = pads)
    """
    rows = np.asarray(rows)
    cols = np.asarray(cols)
    vals = np.asarray(vals, np.float32)

    NT = nblk * T0
    rr = np.zeros((NCORES, 128, NT), dtype=np.float32)
    val = np.zeros((NCORES, 128, NT), dtype=np.float32)
    lanecol = np.zeros((NCORES, NT * 128), dtype=np.int32)

    core_bounds = np.searchsorted(rows, np.arange(NCORES + 1) * ROWS_PER_CORE)
    for c in range(NCORES):
        e0, e1 = core_bounds[c], core_bounds[c + 1]
        r = rows[e0:e1] - c * ROWS_PER_CORE
        bounds = [st for st, _ in plans[c]] + [ROWS_PER_CORE]
        bb = np.searchsorted(r, bounds)
        for b in range(nblk):
            s, e = e0 + bb[b], e0 + bb[b + 1]
            n = e - s
            assert n <= T0 * 128, (c, b, n)
            if n == 0:
                continue
            j = np.arange(n)
            brow = (rows[s:e] - c * ROWS_PER_CORE - plans[c][b][0])
            rr[c, j % 128, b * T0 + j // 128] = brow.astype(np.float32)
            val[c, j % 128, b * T0 + j // 128] = vals[s:e]
            lanecol[c, b * T0 * 128 + j] = cols[s:e]
    return rr, val, lanecol


def _build(nblk, T0, R, rmax):
    import concourse.bacc as bacc
    import concourse.mybir as mybir
    import concourse.tile as tile

    f32 = mybir.dt.float32
    bf16 = mybir.dt.bfloat16

    nc = bacc.Bacc("TRN2", target_bir_lowering=False)

    T2 = 2 * T0
    slab_t = nc.dram_tensor("slab", [128, nblk * T2, 128], bf16,
                            kind="ExternalInput")
    rr_t = nc.dram_tensor("rrcat", [128, nblk * T2], bf16, kind="ExternalInput")
    val_t = nc.dram_tensor("valcat", [128, nblk * T2], bf16,
                           kind="ExternalInput")
    iota_t = nc.dram_tensor("iota_rep", [128, T2, R], bf16,
                            kind="ExternalInput")
    wlpT_t = nc.dram_tensor("wlpT", [D, D], bf16, kind="ExternalInput")
    whpT_t = nc.dram_tensor("whpT", [D, D], bf16, kind="ExternalInput")
    bias_t = nc.dram_tensor("bias", [128, 1], f32, kind="ExternalInput")
    out_t = nc.dram_tensor("out", [nblk, 128, R], bf16, kind="ExternalOutput")

    with tile.TileContext(nc) as tc:
        with (
            tc.tile_pool(name="const", bufs=1) as cpool,
            tc.tile_pool(name="gbuf", bufs=10) as gpool,
            tc.tile_pool(name="abuf", bufs=6) as apool,
            tc.tile_pool(name="cagg", bufs=6) as caggpool,
            tc.tile_pool(name="osb", bufs=4) as opool,
            tc.tile_pool(name="psagg", bufs=2, space="PSUM") as psagg,
            tc.tile_pool(name="ps2", bufs=2, space="PSUM") as ps2,
        ):
            rr_sb = cpool.tile_from(rr_t[:], name="rrcat")
            val_sb = cpool.tile_from(val_t[:], name="valcat")
            iota_sb = cpool.tile_from(iota_t[:], name="iota_rep")
            wlpT = cpool.tile_from(wlpT_t[:], name="wlpT")
            whpT = cpool.tile_from(whpT_t[:], name="whpT")
            bias = cpool.tile_from(bias_t[:], name="bias")

            for b in range(nblk):
                w = rmax[b]
                g = gpool.tile([128, T2, 128], bf16, tag="g")
                nc.sync.dma_start(g[:], slab_t[:, b * T2 : (b + 1) * T2, :])

                # A tiles for all 2*T0 tiles of the block, tile-major
                # [128, T2, w] so the matmul's moving columns are contiguous.
                a_t = apool.tile([128, T2, R], bf16, tag="A")
                rr_bc = rr_sb[:, b * T2 : (b + 1) * T2, None].broadcast_to(
                    [128, T2, w])
                val_bc = val_sb[:, b * T2 : (b + 1) * T2, None].broadcast_to(
                    [128, T2, w])
                nc.vector.tensor_tensor(
                    a_t[:, :, :w], iota_sb[:, :, :w], rr_bc,
                    mybir.AluOpType.is_equal)
                nc.vector.tensor_tensor(
                    a_t[:, :, :w], a_t[:, :, :w], val_bc,
                    mybir.AluOpType.mult)

                caggs = {}
                for s, off in (("lp", 0), ("hp", T0)):
                    aggT = psagg.tile([128, R], f32, tag=f"aggT_{s}")
                    for t in range(T0):
                        nc.tensor.matmul(
                            aggT[:, :w],
                            g[:, off + t, :],
                            a_t[:, off + t, :w],
                            start=(t == 0),
                            stop=(t == T0 - 1),
                        )
                    cagg = caggpool.tile([128, R], bf16, tag=f"cagg_{s}")
                    nc.scalar.copy(cagg[:, :w], aggT[:, :w])
                    caggs[s] = cagg

                psum2 = ps2.tile([128, R], f32, tag="psum2")
                nc.tensor.matmul(psum2[:, :w], wlpT[:], caggs["lp"][:, :w],
                                 start=True, stop=False)
                nc.tensor.matmul(psum2[:, :w], whpT[:], caggs["hp"][:, :w],
                                 start=False, stop=True)
                osb = opool.tile([128, R], bf16, tag="osb")
                nc.scalar.activation(
                    osb[:, :w], psum2[:, :w],
                    mybir.ActivationFunctionType.Relu,
                    bias=bias[:, 0:1],
                )
                nc.scalar.dma_start(out_t[b, :, :w], osb[:, :w])

    nc.compile()
    return nc


def kernel(x, lp_rows, lp_cols, lp_vals, hp_rows, hp_cols, hp_vals,
           W_LP, W_HP, bias, alpha_raw):
    import ml_dtypes
    from concourse.bass_utils import run_bass_kernel_spmd

    x = np.asarray(x, dtype=np.float32)
    alpha = 1.0 / (1.0 + np.exp(-float(np.asarray(alpha_raw).reshape(-1)[0])))

    T0 = int(os.environ.get("K2_T0", "6"))
    R = int(os.environ.get("K2_R", "64"))

    plans, nblk = _plan_blocks(lp_rows, hp_rows, T0 * 128, R)
    rmax = tuple(max(plans[c][b][1] - plans[c][b][0] for c in range(NCORES))
                 for b in range(nblk))
    rr_lp, val_lp, lc_lp = _prep_set(
        lp_rows, lp_cols, np.asarray(lp_vals, np.float32) * np.float32(alpha),
        plans, nblk, T0)
    rr_hp, val_hp, lc_hp = _prep_set(
        hp_rows, hp_cols,
        np.asarray(hp_vals, np.float32) * np.float32(1.0 - alpha),
        plans, nblk, T0)

    bf = ml_dtypes.bfloat16
    xbf = np.ascontiguousarray(x.astype(bf))
    wlpT = np.ascontiguousarray(np.asarray(W_LP, np.float32).T.astype(bf))
    whpT = np.ascontiguousarray(np.asarray(W_HP, np.float32).T.astype(bf))
    bias_col = np.ascontiguousarray(np.asarray(bias, np.float32).reshape(128, 1))
    T2 = 2 * T0
    iota_rep = np.ascontiguousarray(np.broadcast_to(
        np.arange(R, dtype=np.float32)[None, None, :],
        (128, T2, R)).astype(bf))

    NT = nblk * T0

    def cat_meta(m_lp, m_hp):
        # [128, nblk*T0] x2 -> [128, nblk*2T0] with per-block lp then hp
        a = m_lp.reshape(128, nblk, T0)
        b = m_hp.reshape(128, nblk, T0)
        return np.ascontiguousarray(
            np.concatenate([a, b], axis=2).reshape(128, nblk * T2).astype(bf))

    def slabcat(lcl, lch):
        # lane cols [NT*128] x2 -> gathered x rows in [128, nblk*2T0, 128]
        a = lcl.reshape(nblk, T0 * 128)
        b = lch.reshape(nblk, T0 * 128)
        lanes = np.concatenate([a, b], axis=1).reshape(nblk * T2, 128)
        g = xbf[lanes]                         # [nblk*2T0, 128, 128]
        return np.ascontiguousarray(g.transpose(1, 0, 2))

    in_maps = []
    for c in range(NCORES):
        m = {
            "slab": slabcat(lc_lp[c], lc_hp[c]),
            "rrcat": cat_meta(rr_lp[c], rr_hp[c]),
            "valcat": cat_meta(val_lp[c], val_hp[c]),
            "iota_rep": iota_rep, "wlpT": wlpT, "whpT": whpT,
            "bias": bias_col,
        }
        in_maps.append(m)

    key = (nblk, T0, R, rmax)
    trace = bool(int(os.environ.get("KERNEL_TRACE", "0")))
    res = None
    last_exc = None
    # Rarely the device comes up in a bad state and an execution fails; retry.
    for attempt in range(3):
        if key not in _COMPILED:
            _COMPILED[key] = _build(*key)
        try:
            res = run_bass_kernel_spmd(
                _COMPILED[key], in_maps, list(range(NCORES)), trace=trace)
            break
        except Exception as e:  # noqa: BLE001
            last_exc = e
    if res is None:
        raise last_exc
    kernel.last_result = res

    out = np.empty((N_NODES, D), dtype=np.float32)
    for c in range(NCORES):
        oc = np.asarray(res.results[c]["out"], dtype=np.float32)
        base = c * ROWS_PER_CORE
        for b, (r0, r1) in enumerate(plans[c]):
            out[base + r0 : base + r1, :] = oc[b, :, : r1 - r0].T
    return out
